# revision 24
# baseline (speedup 1.0000x reference)
"""DeepSeekMoE (E=8, top-2) forward as a Trainium2 Bass kernel.

Strategy: 2 expert-groups x 4 F-slices over the 8 cores.
  - Host computes gate logits to derive the discrete routing (top-2
    expert ids per token) and the top-2 softmax combine weights.
  - Experts are split into 2 groups of 4, chosen so the groups' token
    loads balance; cores (g, s) with s in 0..3 hold the F-slice
    [s*1024, (s+1)*1024) of group g's four experts (16.8 MB bf16,
    SBUF-resident).
  - Each group's tokens form one stream, grouped by expert (rank-sorted
    by count, padded per-rank to the max across groups so both groups
    share one SPMD chunk schedule). Every core of a group processes the
    whole stream over its F-slice:
      h_c = silu(x @ W1[e][:, sl] + b1[e][sl]);  y_c = h_c @ W2[e][sl, :]
  - Host sums the 4 F-slice partials per group, applies the combine
    weight + b2, and scatter-adds into the output.

Device layout per chunk of <=512 tokens (single expert per chunk):
  M1: W1 stationary [128(D-k), 128(f)] x moving [128(D-k), W] -> h in
      [f-partition, token-free]; silu on Scalar.
  M2: W2 stationary [128(F-k), 128(d)] x h moving [128(F-k), W] -> y in
      [d-partition, token-free]; PSUM -> f16 evict on Vector.
All DRAM buffers are host-packed so every DMA line is contiguous per
partition (7-16 KB): x/y use per-chunk [128, 8*W] blocks, weights are
[128, e*8K..] flat. Queues: x on Sync, weights on Activation, y on
GpSimd — three independent queues, no head-of-line blocking.
All matmuls bf16 (fp32 PSUM).
"""

import os
import sys

import numpy as np

sys.path.insert(0, "/opt/trn_rl_repo")

import ml_dtypes  # noqa: E402

import concourse.bass as bass  # noqa: E402
import concourse.tile as tile  # noqa: E402
from concourse import mybir  # noqa: E402
from concourse.bass import ds, ts  # noqa: E402
from concourse.bass_utils import run_bass_kernel_spmd  # noqa: E402

NUM_EXPERTS = 8
TOP_K = 2
D = 1024
F = 4096
N_GROUPS = 2
N_SLICES = 4
EPG = NUM_EXPERTS // N_GROUPS  # experts per group
FSL = F // N_SLICES  # 1024: per-core F-slice
CHUNK_MAX = 512
N_D = D // 128  # 8 contraction tiles for matmul1 / output cols for matmul2
N_F = FSL // 128  # 8 F-slice tiles
BF16 = mybir.dt.bfloat16
F16 = mybir.dt.float16
F32 = mybir.dt.float32

_AF = mybir.ActivationFunctionType


def _legalize_waits(nc: bass.Bass, max_waits: int = 1) -> int:
    """This container's walrus build can encode at most ONE semaphore wait
    per instruction ("Too many sync wait commands" otherwise — even the
    repo's own Tile kernels trip it). Hoist extra waits onto same-engine
    NoOps inserted immediately before the offending instruction."""
    n_fix = 0
    for f in nc.m.functions:
        for blk in f.blocks:
            idx = 0
            while idx < len(blk.instructions):
                inst = blk.instructions[idx]
                si = inst.sync_info
                if (
                    si is not None
                    and si.on_wait
                    and len(si.on_wait) > max_waits
                    and type(inst).__name__ != "InstNoOp"
                ):
                    waits = list(si.on_wait)
                    keep, extra = waits[-max_waits:], waits[:-max_waits]
                    for j, w in enumerate(extra):
                        nop = mybir.InstNoOp(
                            name=f"LGW-{nc.next_id()}", ins=[], outs=[]
                        )
                        nop.engine = inst.engine
                        nop.sync_info = mybir.SyncInfo(on_wait=[w], on_update=[])
                        nc.register_instruction(nop)
                        blk.instructions.insert(idx + j, nop)
                    inst.sync_info = mybir.SyncInfo(
                        on_wait=keep, on_update=list(si.on_update)
                    )
                    idx += len(extra) + 1
                    n_fix += 1
                else:
                    idx += 1
    return n_fix


def _build_program(chunks: list[tuple[int, int, int]], Ttot: int, use_b1: bool):
    """Trace the single SPMD program run by all 8 cores.

    chunks: list of (expert_slot 0..3, stream_offset, width<=512).
    Ttot: total stream length (sum of widths).
    """
    WPE = N_D * FSL  # 8192 flat weight columns per expert (both tensors)

    nc = bass.Bass(debug=False)
    xp_d = nc.declare_dram_parameter("xp", [128, N_D * Ttot], BF16,
                                     isOutput=False)
    w1_d = nc.declare_dram_parameter("w1", [128, EPG * WPE], BF16,
                                     isOutput=False)
    w2_d = nc.declare_dram_parameter("w2", [128, EPG * WPE], BF16,
                                     isOutput=False)
    if use_b1:
        b1_d = nc.declare_dram_parameter("b1", [128, EPG * N_F], F32,
                                         isOutput=False)
    y_d = nc.declare_dram_parameter("y", [128, N_D * Ttot], F16, isOutput=True)

    with tile.TileContext(nc) as tc:
        with (
            tc.tile_pool(name="consts", bufs=1) as consts,
            tc.tile_pool(name="xin", bufs=3) as xin,
            tc.tile_pool(name="hbuf", bufs=2) as hbuf,
            tc.tile_pool(name="ybuf", bufs=2) as ybuf,
            tc.tile_pool(name="ps1p", bufs=2, space="PSUM") as ps1p,
            tc.tile_pool(name="ps2p", bufs=2, space="PSUM") as ps2p,
        ):
            # ---- resident weights ----
            w1_sb = consts.tile([128, EPG * WPE], BF16)
            w2_sb = consts.tile([128, EPG * WPE], BF16)
            if use_b1:
                b1_sb = consts.tile([128, EPG * N_F], F32)
                nc.scalar.dma_start(b1_sb[:], b1_d[:])

            # HAM warm-up: a few matmuls on memset data run while the first
            # x/W DMAs are in flight so real matmuls start near 2.4 GHz
            # (kept short — chunk 0's matmuls queue behind them).
            warm_sb = consts.tile([128, CHUNK_MAX], BF16)
            nc.vector.memset(warm_sb[:], 1.0)
            for _ in range(5):
                ps_w = ps1p.tile([128, 2, CHUNK_MAX], F32, tag="ps1")
                for j in range(2):
                    nc.tensor.matmul(
                        ps_w[:, j, :], warm_sb[:, 0:128], warm_sb[:],
                        start=True, stop=True,
                    )

            # Per-queue DMA bandwidth is only ~100-150 GB/s, and a full
            # queue backpressures the issuing engine's instruction stream.
            # So: the Activation queue carries ONLY w1[e0] (+ half of x0)
            # so silu is never stuck behind weight issues; GpSimd carries
            # w2[e0] + w1[e1] before the y stream starts; every other
            # weight half trickles one-per-chunk on the Sync queue behind
            # the x prefetches (emitted inside the chunk loop below).
            QF = WPE // 4
            HF = WPE // 2

            x_tiles = {}

            def fetch_x(ci, split=False):
                if ci >= len(chunks):
                    return
                _, s0, W = chunks[ci]
                x_c = xin.tile([128, N_D * CHUNK_MAX], BF16, tag="x")
                hw = N_D * W // 2
                if split:  # halve startup latency across two queues
                    nc.sync.dma_start(x_c[:, :hw], xp_d[:, ds(N_D * s0, hw)])
                    nc.gpsimd.dma_start(
                        x_c[:, ds(hw, N_D * W - hw)],
                        xp_d[:, ds(N_D * s0 + hw, N_D * W - hw)],
                    )
                else:
                    nc.sync.dma_start(
                        x_c[:, : N_D * W], xp_d[:, ds(N_D * s0, N_D * W)]
                    )
                x_tiles[ci] = x_c

            # startup set spread evenly: scalar w1 q0/q2, sync x0h1 + w1
            # q1/q3, gpsimd x0h2 + all of w2[e0] + w1[e1]
            nc.scalar.dma_start(w1_sb[:, ds(0, QF)], w1_d[:, ds(0, QF)])
            fetch_x(0, split=True)
            nc.scalar.dma_start(w1_sb[:, ds(2 * QF, QF)], w1_d[:, ds(2 * QF, QF)])
            nc.sync.dma_start(w1_sb[:, ds(1 * QF, QF)], w1_d[:, ds(1 * QF, QF)])
            nc.sync.dma_start(w1_sb[:, ds(3 * QF, QF)], w1_d[:, ds(3 * QF, QF)])
            for q in range(4):
                nc.gpsimd.dma_start(
                    w2_sb[:, ds(q * QF, QF)], w2_d[:, ds(q * QF, QF)]
                )
            fetch_x(1)
            for hh in range(2):
                nc.gpsimd.dma_start(
                    w1_sb[:, ds(WPE + hh * HF, HF)],
                    w1_d[:, ds(WPE + hh * HF, HF)],
                )
            # remaining weight halves, one per chunk on the Sync queue
            wq = [("w2", 1, 0), ("w2", 1, 1)]
            for e in range(2, EPG):
                wq += [("w1", e, 0), ("w1", e, 1), ("w2", e, 0), ("w2", e, 1)]

            def issue_next_weight():
                if not wq:
                    return
                kind, e, hh = wq.pop(0)
                sb, dr = (w1_sb, w1_d) if kind == "w1" else (w2_sb, w2_d)
                nc.sync.dma_start(
                    sb[:, ds(e * WPE + hh * HF, HF)],
                    dr[:, ds(e * WPE + hh * HF, HF)],
                )

            # ---- main pipeline over token chunks ----
            for ci, (e, s0, W) in enumerate(chunks):
                fetch_x(ci + 2)
                if ci >= 1:
                    issue_next_weight()
                x_c = x_tiles.pop(ci)

                # matmul1 + silu: h tiles [128(F-slice), W]. PSUM tiles are
                # 2-bank pairs (halves the tile-alloc count -> shorter
                # teardown); f-tile f uses slot f%2.
                h_c = hbuf.tile([128, N_F * CHUNK_MAX], BF16, tag="h")
                for f in range(N_F):
                    if f % 2 == 0:
                        ps1 = ps1p.tile([128, 2, CHUNK_MAX], F32, tag="ps1")
                    pf = ps1[:, f % 2, :W]
                    for d in range(N_D):
                        nc.tensor.matmul(
                            pf,
                            w1_sb[:, ds(e * WPE + f * D + d * 128, 128)],
                            x_c[:, ds(d * W, W)],
                            start=(d == 0),
                            stop=(d == N_D - 1),
                        )
                    if use_b1:
                        nc.scalar.activation(
                            h_c[:, ds(f * W, W)], pf, _AF.Silu,
                            bias=b1_sb[:, e * N_F + f : e * N_F + f + 1],
                        )
                    else:
                        nc.scalar.activation(h_c[:, ds(f * W, W)], pf, _AF.Silu)

                # matmul2: y tiles [128(D-col), W], f16 partials out.
                # y DMAs (GpSimd queue) go out in two halves so the final
                # drain after the last matmul is short.
                y_c = ybuf.tile([128, N_D * CHUNK_MAX], F16, tag="y")
                for n in range(N_D):
                    if n % 2 == 0:
                        ps2 = ps2p.tile([128, 2, CHUNK_MAX], F32, tag="ps2")
                    pn = ps2[:, n % 2, :W]
                    for k in range(N_F):
                        nc.tensor.matmul(
                            pn,
                            w2_sb[:, ds(e * WPE + n * FSL + k * 128, 128)],
                            h_c[:, ds(k * W, W)],
                            start=(k == 0),
                            stop=(k == N_F - 1),
                        )
                    nc.vector.tensor_copy(y_c[:, ds(n * W, W)], pn)
                    if n == N_D // 2 - 1:
                        nc.gpsimd.dma_start(
                            y_d[:, ds(N_D * s0, N_D * W // 2)],
                            y_c[:, : N_D * W // 2],
                        )
                nc.gpsimd.dma_start(
                    y_d[:, ds(N_D * s0 + N_D * W // 2, N_D * W // 2)],
                    y_c[:, ds(N_D * W // 2, N_D * W // 2)],
                )

    _legalize_waits(nc)
    return nc


def _enable_tracing_shims():
    """Profiling-only (MOE_KERNEL_TRACE=1): install the NTFF profile hook
    that the boot skips when antenv.axon_hooks is missing, and stub out the
    artifact upload (no network in this sandbox)."""
    import types

    try:
        import antenv.axon_hooks  # noqa: F401
    except ImportError:
        try:
            import antenv
            from trn_agent_boot.trn_boot import _ntff_profile_via_ctypes

            hook = _ntff_profile_via_ctypes("/opt/axon/libaxon_pjrt.so")
            mod = types.ModuleType("antenv.axon_hooks")
            mod._hook = hook
            mod.get_axon_ntff_profile_hook = lambda: mod._hook
            mod.set_axon_ntff_profile_hook = lambda h: setattr(mod, "_hook", h)
            sys.modules["antenv.axon_hooks"] = mod
            antenv.axon_hooks = mod
        except Exception as e:  # pragma: no cover
            print(f"NTFF hook install failed: {e}", file=sys.stderr)

    import concourse.bass_utils as _bu

    _bu.upload_artifacts = lambda tmpdir: f"local:{tmpdir}"


def _pack_tokens(xs: np.ndarray, chunks) -> np.ndarray:
    """[Ttot, D] f32 -> [128, 8*Ttot] bf16 per-chunk blocks (d-major)."""
    blocks = []
    for _, s0, W in chunks:
        blk = xs[s0 : s0 + W].reshape(W, N_D, 128).transpose(2, 1, 0)
        blocks.append(blk.reshape(128, N_D * W))
    return np.ascontiguousarray(np.concatenate(blocks, axis=1)).astype(
        ml_dtypes.bfloat16
    )


def kernel(**inputs) -> np.ndarray:
    x = np.asarray(inputs["x"], dtype=np.float32)
    gate_w = np.asarray(inputs["gate_w"], dtype=np.float32)
    gate_b = np.asarray(inputs["gate_b"], dtype=np.float32)
    W1 = np.asarray(inputs["W1"], dtype=np.float32)
    b1 = np.asarray(inputs["b1"], dtype=np.float32)
    W2 = np.asarray(inputs["W2"], dtype=np.float32)
    b2 = np.asarray(inputs["b2"], dtype=np.float32)

    B, S, D_ = x.shape
    T = B * S
    xf = x.reshape(T, D_)

    # ---- host: routing + combine weights ----
    logits = xf @ gate_w + gate_b  # [T, E]
    top2 = np.argpartition(-logits, TOP_K - 1, axis=1)[:, :TOP_K]
    lv = np.take_along_axis(logits.astype(np.float64), top2, 1)  # [T, 2]
    ex = np.exp(lv - lv.max(axis=1, keepdims=True))
    tw = ex / ex.sum(axis=1, keepdims=True)  # [T, 2] softmax over the pair

    sel = np.zeros((T, NUM_EXPERTS), dtype=bool)
    sel[np.arange(T)[:, None], top2] = True
    idx_per_e = [np.nonzero(sel[:, e])[0] for e in range(NUM_EXPERTS)]
    counts = np.array([len(i) for i in idx_per_e])

    # Partition experts into 2 groups of 4 minimizing the summed per-rank
    # capacity (each group rank-sorted desc; cap_r = max over groups of
    # the rank-r count). Only C(8,4)/2 = 35 partitions: brute force.
    from itertools import combinations

    all_e = list(range(NUM_EXPERTS))
    best = None
    for comb in combinations(all_e[1:], EPG - 1):
        g1 = sorted([all_e[0], *comb], key=lambda e: -counts[e])
        g2 = sorted(
            [e for e in all_e if e not in g1], key=lambda e: -counts[e]
        )
        cps = [int(max(counts[g1[r]], counts[g2[r]])) for r in range(EPG)]
        if best is None or sum(cps) < sum(best[0]):
            best = (cps, [g1, g2])
    caps, groups = best
    caps = [max(c, 1) for c in caps]
    Ttot = int(sum(caps))

    # shared SPMD chunk schedule: per rank-region, even chunks <= 512
    chunks = []  # (expert_slot, stream_offset, width)
    reg_off = []
    off = 0
    for r in range(EPG):
        reg_off.append(off)
        n_ch = -(-caps[r] // CHUNK_MAX)
        base, rem = divmod(caps[r], n_ch)
        o = off
        for j in range(n_ch):
            w = base + (1 if j < rem else 0)
            chunks.append((r, o, w))
            o += w
        off += caps[r]
    # carve a small final chunk so the post-last-matmul drain
    # (evict + y DMA) is short
    TAIL = 160
    r, o, w = chunks[-1]
    if w > TAIL + 128:
        chunks[-1] = (r, o, w - TAIL)
        chunks.append((r, o + w - TAIL, TAIL))

    use_b1 = bool(np.any(b1 != 0.0))

    # per-group token streams (zero-padded per rank-region), packed
    xp_g = []
    for g in range(N_GROUPS):
        xs = np.zeros((Ttot, D_), dtype=np.float32)
        for r in range(EPG):
            e = groups[g][r]
            idx = idx_per_e[e]
            xs[reg_off[r] : reg_off[r] + len(idx)] = xf[idx]
        xp_g.append(_pack_tokens(xs, chunks))

    def pack_w(mats):
        """list of [1024, 1024] (rows = k*128+p) -> [128, EPG*8192] with
        flat offset e*8192 + c_tile*1024 + k*128 (+col): the 128-col tile
        c of the stationary operand is contiguous per k, so the first
        output tile needs only the first 256 KB of the expert's block."""
        a = np.stack(mats)  # [EPG, 1024(k*128+p), 1024(c_tile*128+col)]
        a = a.reshape(EPG, N_D, 128, N_D, 128).transpose(2, 0, 3, 1, 4)
        return np.ascontiguousarray(a.reshape(128, -1)).astype(
            ml_dtypes.bfloat16
        )

    in_maps = []
    for core in range(NUM_EXPERTS):
        g, s = divmod(core, N_SLICES)
        fsl = slice(s * FSL, (s + 1) * FSL)
        ge = groups[g]
        m = {
            "xp": xp_g[g],
            "w1": pack_w([W1[e][:, fsl] for e in ge]),
            "w2": pack_w([W2[e][fsl, :] for e in ge]),
        }
        if use_b1:
            m["b1"] = np.ascontiguousarray(
                np.concatenate([b1[e][fsl] for e in ge])
                .reshape(EPG * N_F, 128)
                .T
            )
        in_maps.append(m)

    nc = _build_program(chunks, Ttot, use_b1)
    trace = bool(int(os.environ.get("MOE_KERNEL_TRACE", "0")))
    if trace:
        _enable_tracing_shims()
    res = run_bass_kernel_spmd(nc, in_maps, list(range(NUM_EXPERTS)), trace=trace)
    if trace:
        kernel.last_results = res

    # ---- host: combine (sum F-slice partials, unpack, weight, scatter) ----
    out = np.zeros((T, D_), dtype=np.float32)
    for g in range(N_GROUPS):
        ypk = res.results[g * N_SLICES]["y"].astype(np.float32)
        for s in range(1, N_SLICES):
            ypk += res.results[g * N_SLICES + s]["y"].astype(np.float32)
        ysum = np.empty((D_, Ttot), dtype=np.float32)
        for _, s0, W in chunks:
            blk = ypk[:, N_D * s0 : N_D * (s0 + W)].reshape(128, N_D, W)
            ysum[:, s0 : s0 + W] = blk.transpose(1, 0, 2).reshape(D_, W)
        for r in range(EPG):
            e = groups[g][r]
            idx = idx_per_e[e]
            n_e = len(idx)
            if n_e == 0:
                continue
            we = np.where(top2[idx, 0] == e, tw[idx, 0], tw[idx, 1]).astype(
                np.float32
            )
            ye = ysum[:, reg_off[r] : reg_off[r] + n_e].T + b2[e]
            out[idx] += we[:, None] * ye
    return out.reshape(B, S, D_)


# revision 25
# speedup vs baseline: 1.0049x; 1.0049x over previous
"""DeepSeekMoE (E=8, top-2) forward as a Trainium2 Bass kernel.

Strategy: 2 expert-groups x 4 F-slices over the 8 cores.
  - Host computes gate logits to derive the discrete routing (top-2
    expert ids per token) and the top-2 softmax combine weights.
  - Experts are split into 2 groups of 4, chosen so the groups' token
    loads balance; cores (g, s) with s in 0..3 hold the F-slice
    [s*1024, (s+1)*1024) of group g's four experts (16.8 MB bf16,
    SBUF-resident).
  - Each group's tokens form one stream, grouped by expert (rank-sorted
    by count, padded per-rank to the max across groups so both groups
    share one SPMD chunk schedule). Every core of a group processes the
    whole stream over its F-slice:
      h_c = silu(x @ W1[e][:, sl] + b1[e][sl]);  y_c = h_c @ W2[e][sl, :]
  - Host sums the 4 F-slice partials per group, applies the combine
    weight + b2, and scatter-adds into the output.

Device layout per chunk of <=512 tokens (single expert per chunk):
  M1: W1 stationary [128(D-k), 128(f)] x moving [128(D-k), W] -> h in
      [f-partition, token-free]; silu on Scalar.
  M2: W2 stationary [128(F-k), 128(d)] x h moving [128(F-k), W] -> y in
      [d-partition, token-free]; PSUM -> f16 evict on Vector.
All DRAM buffers are host-packed so every DMA line is contiguous per
partition (7-16 KB): x/y use per-chunk [128, 8*W] blocks, weights are
[128, e*8K..] flat. Queues: x on Sync, weights on Activation, y on
GpSimd — three independent queues, no head-of-line blocking.
All matmuls bf16 (fp32 PSUM).
"""

import os
import sys

import numpy as np

sys.path.insert(0, "/opt/trn_rl_repo")

import ml_dtypes  # noqa: E402

import concourse.bass as bass  # noqa: E402
import concourse.tile as tile  # noqa: E402
from concourse import mybir  # noqa: E402
from concourse.bass import ds, ts  # noqa: E402
from concourse.bass_utils import run_bass_kernel_spmd  # noqa: E402

NUM_EXPERTS = 8
TOP_K = 2
D = 1024
F = 4096
N_GROUPS = 2
N_SLICES = 4
EPG = NUM_EXPERTS // N_GROUPS  # experts per group
FSL = F // N_SLICES  # 1024: per-core F-slice
CHUNK_MAX = 512
N_D = D // 128  # 8 contraction tiles for matmul1 / output cols for matmul2
N_F = FSL // 128  # 8 F-slice tiles
BF16 = mybir.dt.bfloat16
F16 = mybir.dt.float16
F32 = mybir.dt.float32

_AF = mybir.ActivationFunctionType


def _legalize_waits(nc: bass.Bass, max_waits: int = 1) -> int:
    """This container's walrus build can encode at most ONE semaphore wait
    per instruction ("Too many sync wait commands" otherwise — even the
    repo's own Tile kernels trip it). Hoist extra waits onto same-engine
    NoOps inserted immediately before the offending instruction."""
    n_fix = 0
    for f in nc.m.functions:
        for blk in f.blocks:
            idx = 0
            while idx < len(blk.instructions):
                inst = blk.instructions[idx]
                si = inst.sync_info
                if (
                    si is not None
                    and si.on_wait
                    and len(si.on_wait) > max_waits
                    and type(inst).__name__ != "InstNoOp"
                ):
                    waits = list(si.on_wait)
                    keep, extra = waits[-max_waits:], waits[:-max_waits]
                    for j, w in enumerate(extra):
                        nop = mybir.InstNoOp(
                            name=f"LGW-{nc.next_id()}", ins=[], outs=[]
                        )
                        nop.engine = inst.engine
                        nop.sync_info = mybir.SyncInfo(on_wait=[w], on_update=[])
                        nc.register_instruction(nop)
                        blk.instructions.insert(idx + j, nop)
                    inst.sync_info = mybir.SyncInfo(
                        on_wait=keep, on_update=list(si.on_update)
                    )
                    idx += len(extra) + 1
                    n_fix += 1
                else:
                    idx += 1
    return n_fix


def _build_program(chunks: list[tuple[int, int, int]], Ttot: int, use_b1: bool):
    """Trace the single SPMD program run by all 8 cores.

    chunks: list of (expert_slot 0..3, stream_offset, width<=512).
    Ttot: total stream length (sum of widths).
    """
    WPE = N_D * FSL  # 8192 flat weight columns per expert (both tensors)

    nc = bass.Bass(debug=False)
    xp_d = nc.declare_dram_parameter("xp", [128, N_D * Ttot], BF16,
                                     isOutput=False)
    w1_d = nc.declare_dram_parameter("w1", [128, EPG * WPE], BF16,
                                     isOutput=False)
    w2_d = nc.declare_dram_parameter("w2", [128, EPG * WPE], BF16,
                                     isOutput=False)
    if use_b1:
        b1_d = nc.declare_dram_parameter("b1", [128, EPG * N_F], F32,
                                         isOutput=False)
    y_d = nc.declare_dram_parameter("y", [128, N_D * Ttot], F16, isOutput=True)

    with tile.TileContext(nc) as tc:
        with (
            tc.tile_pool(name="consts", bufs=1) as consts,
            tc.tile_pool(name="xin", bufs=3) as xin,
            tc.tile_pool(name="hbuf", bufs=2) as hbuf,
            tc.tile_pool(name="ybuf", bufs=2) as ybuf,
            tc.tile_pool(name="ps1p", bufs=2, space="PSUM") as ps1p,
            tc.tile_pool(name="ps2p", bufs=2, space="PSUM") as ps2p,
        ):
            # ---- resident weights ----
            w1_sb = consts.tile([128, EPG * WPE], BF16)
            w2_sb = consts.tile([128, EPG * WPE], BF16)
            if use_b1:
                b1_sb = consts.tile([128, EPG * N_F], F32)
                nc.scalar.dma_start(b1_sb[:], b1_d[:])

            # HAM warm-up: a few matmuls on memset data run while the first
            # x/W DMAs are in flight so real matmuls start near 2.4 GHz
            # (kept short — chunk 0's matmuls queue behind them).
            warm_sb = consts.tile([128, CHUNK_MAX], BF16)
            nc.vector.memset(warm_sb[:], 1.0)
            for _ in range(3):
                ps_w = ps1p.tile([128, 2, CHUNK_MAX], F32, tag="ps1")
                for j in range(2):
                    nc.tensor.matmul(
                        ps_w[:, j, :], warm_sb[:, 0:128], warm_sb[:],
                        start=True, stop=True,
                    )

            # Per-queue DMA bandwidth is only ~100-150 GB/s, and a full
            # queue backpressures the issuing engine's instruction stream.
            # So: the Activation queue carries ONLY w1[e0] (+ half of x0)
            # so silu is never stuck behind weight issues; GpSimd carries
            # w2[e0] + w1[e1] before the y stream starts; every other
            # weight half trickles one-per-chunk on the Sync queue behind
            # the x prefetches (emitted inside the chunk loop below).
            QF = WPE // 4
            HF = WPE // 2

            x_tiles = {}

            def fetch_x(ci, split=False):
                if ci >= len(chunks):
                    return
                _, s0, W = chunks[ci]
                x_c = xin.tile([128, N_D * CHUNK_MAX], BF16, tag="x")
                hw = N_D * W // 2
                if split:  # halve startup latency across two queues
                    nc.sync.dma_start(x_c[:, :hw], xp_d[:, ds(N_D * s0, hw)])
                    nc.gpsimd.dma_start(
                        x_c[:, ds(hw, N_D * W - hw)],
                        xp_d[:, ds(N_D * s0 + hw, N_D * W - hw)],
                    )
                else:
                    nc.sync.dma_start(
                        x_c[:, : N_D * W], xp_d[:, ds(N_D * s0, N_D * W)]
                    )
                x_tiles[ci] = x_c

            # startup set spread evenly: scalar w1 q0/q2, sync x0h1 + w1
            # q1/q3, gpsimd x0h2 + all of w2[e0] + w1[e1]
            nc.scalar.dma_start(w1_sb[:, ds(0, QF)], w1_d[:, ds(0, QF)])
            fetch_x(0, split=True)
            nc.scalar.dma_start(w1_sb[:, ds(2 * QF, QF)], w1_d[:, ds(2 * QF, QF)])
            nc.sync.dma_start(w1_sb[:, ds(1 * QF, QF)], w1_d[:, ds(1 * QF, QF)])
            nc.sync.dma_start(w1_sb[:, ds(3 * QF, QF)], w1_d[:, ds(3 * QF, QF)])
            for q in range(4):
                nc.gpsimd.dma_start(
                    w2_sb[:, ds(q * QF, QF)], w2_d[:, ds(q * QF, QF)]
                )
            fetch_x(1)
            for hh in range(2):
                nc.gpsimd.dma_start(
                    w1_sb[:, ds(WPE + hh * HF, HF)],
                    w1_d[:, ds(WPE + hh * HF, HF)],
                )
            # remaining weight halves, one per chunk on the Sync queue
            wq = [("w2", 1, 0), ("w2", 1, 1)]
            for e in range(2, EPG):
                wq += [("w1", e, 0), ("w1", e, 1), ("w2", e, 0), ("w2", e, 1)]

            def issue_next_weight():
                if not wq:
                    return
                kind, e, hh = wq.pop(0)
                sb, dr = (w1_sb, w1_d) if kind == "w1" else (w2_sb, w2_d)
                nc.sync.dma_start(
                    sb[:, ds(e * WPE + hh * HF, HF)],
                    dr[:, ds(e * WPE + hh * HF, HF)],
                )

            # ---- main pipeline over token chunks ----
            for ci, (e, s0, W) in enumerate(chunks):
                fetch_x(ci + 2)
                if ci >= 1:
                    issue_next_weight()
                x_c = x_tiles.pop(ci)

                # matmul1 + silu: h tiles [128(F-slice), W]. PSUM tiles are
                # 2-bank pairs (halves the tile-alloc count -> shorter
                # teardown); f-tile f uses slot f%2.
                h_c = hbuf.tile([128, N_F * CHUNK_MAX], BF16, tag="h")
                for f in range(N_F):
                    if f % 2 == 0:
                        ps1 = ps1p.tile([128, 2, CHUNK_MAX], F32, tag="ps1")
                    pf = ps1[:, f % 2, :W]
                    for d in range(N_D):
                        nc.tensor.matmul(
                            pf,
                            w1_sb[:, ds(e * WPE + f * D + d * 128, 128)],
                            x_c[:, ds(d * W, W)],
                            start=(d == 0),
                            stop=(d == N_D - 1),
                        )
                    if use_b1:
                        nc.scalar.activation(
                            h_c[:, ds(f * W, W)], pf, _AF.Silu,
                            bias=b1_sb[:, e * N_F + f : e * N_F + f + 1],
                        )
                    else:
                        nc.scalar.activation(h_c[:, ds(f * W, W)], pf, _AF.Silu)

                # matmul2: y tiles [128(D-col), W], f16 partials out.
                # y DMAs (GpSimd queue) go out in two halves so the final
                # drain after the last matmul is short.
                y_c = ybuf.tile([128, N_D * CHUNK_MAX], F16, tag="y")
                for n in range(N_D):
                    if n % 2 == 0:
                        ps2 = ps2p.tile([128, 2, CHUNK_MAX], F32, tag="ps2")
                    pn = ps2[:, n % 2, :W]
                    for k in range(N_F):
                        nc.tensor.matmul(
                            pn,
                            w2_sb[:, ds(e * WPE + n * FSL + k * 128, 128)],
                            h_c[:, ds(k * W, W)],
                            start=(k == 0),
                            stop=(k == N_F - 1),
                        )
                    nc.vector.tensor_copy(y_c[:, ds(n * W, W)], pn)
                    if n == N_D // 2 - 1:
                        nc.gpsimd.dma_start(
                            y_d[:, ds(N_D * s0, N_D * W // 2)],
                            y_c[:, : N_D * W // 2],
                        )
                nc.gpsimd.dma_start(
                    y_d[:, ds(N_D * s0 + N_D * W // 2, N_D * W // 2)],
                    y_c[:, ds(N_D * W // 2, N_D * W // 2)],
                )

    _legalize_waits(nc)
    return nc


def _enable_tracing_shims():
    """Profiling-only (MOE_KERNEL_TRACE=1): install the NTFF profile hook
    that the boot skips when antenv.axon_hooks is missing, and stub out the
    artifact upload (no network in this sandbox)."""
    import types

    try:
        import antenv.axon_hooks  # noqa: F401
    except ImportError:
        try:
            import antenv
            from trn_agent_boot.trn_boot import _ntff_profile_via_ctypes

            hook = _ntff_profile_via_ctypes("/opt/axon/libaxon_pjrt.so")
            mod = types.ModuleType("antenv.axon_hooks")
            mod._hook = hook
            mod.get_axon_ntff_profile_hook = lambda: mod._hook
            mod.set_axon_ntff_profile_hook = lambda h: setattr(mod, "_hook", h)
            sys.modules["antenv.axon_hooks"] = mod
            antenv.axon_hooks = mod
        except Exception as e:  # pragma: no cover
            print(f"NTFF hook install failed: {e}", file=sys.stderr)

    import concourse.bass_utils as _bu

    _bu.upload_artifacts = lambda tmpdir: f"local:{tmpdir}"


def _pack_tokens(xs: np.ndarray, chunks) -> np.ndarray:
    """[Ttot, D] f32 -> [128, 8*Ttot] bf16 per-chunk blocks (d-major)."""
    blocks = []
    for _, s0, W in chunks:
        blk = xs[s0 : s0 + W].reshape(W, N_D, 128).transpose(2, 1, 0)
        blocks.append(blk.reshape(128, N_D * W))
    return np.ascontiguousarray(np.concatenate(blocks, axis=1)).astype(
        ml_dtypes.bfloat16
    )


def kernel(**inputs) -> np.ndarray:
    x = np.asarray(inputs["x"], dtype=np.float32)
    gate_w = np.asarray(inputs["gate_w"], dtype=np.float32)
    gate_b = np.asarray(inputs["gate_b"], dtype=np.float32)
    W1 = np.asarray(inputs["W1"], dtype=np.float32)
    b1 = np.asarray(inputs["b1"], dtype=np.float32)
    W2 = np.asarray(inputs["W2"], dtype=np.float32)
    b2 = np.asarray(inputs["b2"], dtype=np.float32)

    B, S, D_ = x.shape
    T = B * S
    xf = x.reshape(T, D_)

    # ---- host: routing + combine weights ----
    logits = xf @ gate_w + gate_b  # [T, E]
    top2 = np.argpartition(-logits, TOP_K - 1, axis=1)[:, :TOP_K]
    lv = np.take_along_axis(logits.astype(np.float64), top2, 1)  # [T, 2]
    ex = np.exp(lv - lv.max(axis=1, keepdims=True))
    tw = ex / ex.sum(axis=1, keepdims=True)  # [T, 2] softmax over the pair

    sel = np.zeros((T, NUM_EXPERTS), dtype=bool)
    sel[np.arange(T)[:, None], top2] = True
    idx_per_e = [np.nonzero(sel[:, e])[0] for e in range(NUM_EXPERTS)]
    counts = np.array([len(i) for i in idx_per_e])

    # Partition experts into 2 groups of 4 minimizing the summed per-rank
    # capacity (each group rank-sorted desc; cap_r = max over groups of
    # the rank-r count). Only C(8,4)/2 = 35 partitions: brute force.
    from itertools import combinations

    all_e = list(range(NUM_EXPERTS))
    best = None
    for comb in combinations(all_e[1:], EPG - 1):
        g1 = sorted([all_e[0], *comb], key=lambda e: -counts[e])
        g2 = sorted(
            [e for e in all_e if e not in g1], key=lambda e: -counts[e]
        )
        cps = [int(max(counts[g1[r]], counts[g2[r]])) for r in range(EPG)]
        if best is None or sum(cps) < sum(best[0]):
            best = (cps, [g1, g2])
    caps, groups = best
    caps = [max(c, 1) for c in caps]
    Ttot = int(sum(caps))

    # shared SPMD chunk schedule: per rank-region, even chunks <= 512
    chunks = []  # (expert_slot, stream_offset, width)
    reg_off = []
    off = 0
    for r in range(EPG):
        reg_off.append(off)
        n_ch = -(-caps[r] // CHUNK_MAX)
        base, rem = divmod(caps[r], n_ch)
        o = off
        for j in range(n_ch):
            w = base + (1 if j < rem else 0)
            chunks.append((r, o, w))
            o += w
        off += caps[r]
    # carve a small final chunk so the post-last-matmul drain
    # (evict + y DMA) is short
    TAIL = 160
    r, o, w = chunks[-1]
    if w > TAIL + 128:
        chunks[-1] = (r, o, w - TAIL)
        chunks.append((r, o + w - TAIL, TAIL))

    use_b1 = bool(np.any(b1 != 0.0))

    # per-group token streams (zero-padded per rank-region), packed
    xp_g = []
    for g in range(N_GROUPS):
        xs = np.zeros((Ttot, D_), dtype=np.float32)
        for r in range(EPG):
            e = groups[g][r]
            idx = idx_per_e[e]
            xs[reg_off[r] : reg_off[r] + len(idx)] = xf[idx]
        xp_g.append(_pack_tokens(xs, chunks))

    def pack_w(mats):
        """list of [1024, 1024] (rows = k*128+p) -> [128, EPG*8192] with
        flat offset e*8192 + c_tile*1024 + k*128 (+col): the 128-col tile
        c of the stationary operand is contiguous per k, so the first
        output tile needs only the first 256 KB of the expert's block."""
        a = np.stack(mats)  # [EPG, 1024(k*128+p), 1024(c_tile*128+col)]
        a = a.reshape(EPG, N_D, 128, N_D, 128).transpose(2, 0, 3, 1, 4)
        return np.ascontiguousarray(a.reshape(128, -1)).astype(
            ml_dtypes.bfloat16
        )

    in_maps = []
    for core in range(NUM_EXPERTS):
        g, s = divmod(core, N_SLICES)
        fsl = slice(s * FSL, (s + 1) * FSL)
        ge = groups[g]
        m = {
            "xp": xp_g[g],
            "w1": pack_w([W1[e][:, fsl] for e in ge]),
            "w2": pack_w([W2[e][fsl, :] for e in ge]),
        }
        if use_b1:
            m["b1"] = np.ascontiguousarray(
                np.concatenate([b1[e][fsl] for e in ge])
                .reshape(EPG * N_F, 128)
                .T
            )
        in_maps.append(m)

    nc = _build_program(chunks, Ttot, use_b1)
    trace = bool(int(os.environ.get("MOE_KERNEL_TRACE", "0")))
    if trace:
        _enable_tracing_shims()
    res = run_bass_kernel_spmd(nc, in_maps, list(range(NUM_EXPERTS)), trace=trace)
    if trace:
        kernel.last_results = res

    # ---- host: combine (sum F-slice partials, unpack, weight, scatter) ----
    out = np.zeros((T, D_), dtype=np.float32)
    for g in range(N_GROUPS):
        ypk = res.results[g * N_SLICES]["y"].astype(np.float32)
        for s in range(1, N_SLICES):
            ypk += res.results[g * N_SLICES + s]["y"].astype(np.float32)
        ysum = np.empty((D_, Ttot), dtype=np.float32)
        for _, s0, W in chunks:
            blk = ypk[:, N_D * s0 : N_D * (s0 + W)].reshape(128, N_D, W)
            ysum[:, s0 : s0 + W] = blk.transpose(1, 0, 2).reshape(D_, W)
        for r in range(EPG):
            e = groups[g][r]
            idx = idx_per_e[e]
            n_e = len(idx)
            if n_e == 0:
                continue
            we = np.where(top2[idx, 0] == e, tw[idx, 0], tw[idx, 1]).astype(
                np.float32
            )
            ye = ysum[:, reg_off[r] : reg_off[r] + n_e].T + b2[e]
            out[idx] += we[:, None] * ye
    return out.reshape(B, S, D_)
